# revision 1
# baseline (speedup 1.0000x reference)
"""Binarized Conv1d + BatchNorm1d (training mode) on 8 TRN2 NeuronCores.

Reference computation:
    bx  = sign(x)          [B=16, Cin=128, L=8192]
    bw  = sign(weight)     [Cout=128, Cin=128, K=5]
    out = conv1d(bx, bw, stride=1, pad=2) + bias
    out = (out - mean(out, (B,L))) * rsqrt(var(out, (B,L)) + 1e-5)

Sharding: data-parallel over batch, 2 batches per core.  Weights are
replicated.  Per-channel BN statistics are combined with a tiny
all-reduce ([128,2] f32: mean and E[x^2] of the local shard).

The conv bias cancels exactly inside training-mode BatchNorm
((conv + b) - mean(conv + b) == conv - mean(conv)), so it is ignored.

Kernel structure per core:
  - sign(weight) -> bf16, PE-transpose each tap to [ci, co] stationary tiles
  - stream x in 1 MiB chunks, sign -> bf16 padded row [128, 8196]
  - conv = 5 accumulated bf16 matmuls per [128, 512] PSUM tile
    (sign values are exact in bf16; products are +-1/0 accumulated in
    f32 PSUM, so the conv result is exact integers)
  - bn_stats on each PSUM tile, PSUM -> SBUF copy (conv kept resident
    in SBUF: 64 KiB/partition, avoids a DRAM round-trip)
  - bn_aggr -> pack (mean, E[x^2])/8 -> AllReduce(add) over 8 cores
  - normalize in one tensor_scalar pass, DMA out
"""

import os
import sys

import numpy as np

# concourse is normally importable from the axon site; fall back to the
# staged repo copies if not
try:
    import concourse  # noqa: F401
except ImportError:
    for _p in ("/opt/trn_rl_repo", "/root/.axon_site/_ro/trn_rl_repo"):
        if os.path.isdir(_p):
            sys.path.insert(0, _p)
            break

B = 16
B_LOC = 2
CI = 128
CO = 128
L = 8192
K = 5
PAD = 2
EPS = 1e-5
N_CORES = 8
FREE = 512          # PSUM tile free dim (one bank of f32)
NT = L // FREE      # 16 conv tiles per batch row
XCH = 2048          # x / out DMA chunk columns (1 MiB per transfer)

_CACHE = {}


def _build_nc():
    import concourse.bacc as bacc
    import concourse.bass as bass
    import concourse.tile as tile
    from concourse import mybir
    from concourse.masks import make_identity

    f32 = mybir.dt.float32
    bf16 = mybir.dt.bfloat16
    Sign = mybir.ActivationFunctionType.Sign
    Sqrt = mybir.ActivationFunctionType.Sqrt
    Copy = mybir.ActivationFunctionType.Copy

    nc = bacc.Bacc("TRN2", target_bir_lowering=False, debug=False, num_devices=N_CORES)

    x = nc.declare_dram_parameter("x", [B_LOC, CI, L], f32, isOutput=False)
    w = nc.declare_dram_parameter("weight", [CO, CI, K], f32, isOutput=False)
    out = nc.declare_dram_parameter("out", [B_LOC, CO, L], f32, isOutput=True)

    with tile.TileContext(nc) as tc:
        with (
            tc.tile_pool(name="singles", bufs=1) as singles,
            tc.tile_pool(name="xin", bufs=1) as xin,
            tc.tile_pool(name="bxp", bufs=2) as bxp_pool,
            tc.tile_pool(name="psum", bufs=8, space="PSUM") as psum,
            tc.tile_pool(name="dram", bufs=2, space="DRAM") as dram,
        ):
            # ---- weight + first x chunk DMAs issued before anything else ----
            wf32 = singles.tile([CO, CI, K], f32)
            nc.sync.dma_start(out=wf32, in_=w[:, :, :])
            xts = []
            for b in range(B_LOC):
                xts.append(
                    xin.tile([CI, L], f32, tag=f"xt{b}", name=f"xt{b}")
                )
            nc.sync.dma_start(out=xts[0][:, 0:512], in_=x[0, :, 0:512])

            # ---- warm-up collective ----
            # The first collective in a NEFF pays cross-core rendezvous +
            # setup (~50 us observed).  Fire a tiny dummy AllGather
            # immediately so that cost overlaps with the conv phase; the
            # real stats AllReduce later then runs at its steady-state
            # floor.  (AllGather has the lowest floor of the collectives.)
            warm_sb = singles.tile([1, 8], f32)
            nc.vector.memset(warm_sb, 0.0)
            warm_in = dram.tile([1, 8], f32)
            warm_out = dram.tile([N_CORES, 8], f32)
            nc.gpsimd.dma_start(out=warm_in, in_=warm_sb)
            nc.gpsimd.collective_compute(
                "AllGather",
                mybir.AluOpType.bypass,
                replica_groups=[list(range(N_CORES))],
                ins=[warm_in[:].opt()],
                outs=[warm_out[:].opt()],
            )

            # ---- weights: sign -> bf16, transpose each tap to [ci, co] ----
            ident = singles.tile([128, 128], bf16)
            make_identity(nc, ident)

            wsgn = singles.tile([CO, CI, K], bf16)
            nc.scalar.activation(out=wsgn, in_=wf32, func=Sign)

            wT = singles.tile([CI, K, CO], bf16)  # stationary tiles per tap
            for k in range(K):
                pw = psum.tile([CI, CO], bf16, tag="pt")
                nc.tensor.transpose(pw, wsgn[:, :, k], ident)
                nc.vector.tensor_copy(out=wT[:, k, :], in_=pw)

            # ---- conv + local stats ----
            # conv output kept resident in SBUF: [128 co, B_LOC * L] f32
            conv_sb = singles.tile([CO, B_LOC, L], f32)
            stats = singles.tile([CO, B_LOC * NT, 6], f32)

            # ramped DMA chunks: small first chunk (already issued above
            # for b=0) so the first matmuls start early, small last chunks
            # so the stats finish right behind the last sign; sign emitted
            # per <=1024 cols so matmuls chase the conversion closely
            # tile t's taps need sign cols up to 512*t + 516, i.e. 4 cols
            # into the NEXT chunk -- keep early chunks small so the first
            # matmuls are not gated on a big second chunk
            CHUNK_SCHED = [
                [512, 512, 1024, 2048, 2048, 2048],
                [2048, 2048, 2048, 1024, 512, 512],
            ]
            for b in range(B_LOC):
                bxp = bxp_pool.tile([CI, L + 2 * PAD], bf16)
                nc.vector.memset(bxp[:, 0:PAD], 0.0)
                nc.vector.memset(bxp[:, L + PAD : L + 2 * PAD], 0.0)
                # one staging tile per batch, written once in disjoint
                # chunks -> no DMA ever needs a buffer-reuse wait (HW-queue
                # DMAs only support a single sync wait)
                xt = xts[b]
                off = 0
                for ci_, ch in enumerate(CHUNK_SCHED[b]):
                    if not (b == 0 and ci_ == 0):  # first chunk pre-issued
                        nc.sync.dma_start(
                            out=xt[:, off : off + ch],
                            in_=x[b, :, off : off + ch],
                        )
                    s = off
                    while s < off + ch:
                        sw = min(1024, off + ch - s)
                        nc.scalar.activation(
                            out=bxp[:, PAD + s : PAD + s + sw],
                            in_=xt[:, s : s + sw],
                            func=Sign,
                        )
                        s += sw
                    off += ch
                for t in range(NT):
                    pt = psum.tile([CO, FREE], f32, tag="pt")
                    for k in range(K):
                        nc.tensor.matmul(
                            pt,
                            lhsT=wT[:, k, :],
                            rhs=bxp[:, t * FREE + k : t * FREE + k + FREE],
                            start=(k == 0),
                            stop=(k == K - 1),
                        )
                    nc.vector.bn_stats(out=stats[:, b * NT + t, :], in_=pt)
                    nc.scalar.activation(
                        out=conv_sb[:, b, t * FREE : (t + 1) * FREE], in_=pt, func=Copy
                    )

            # ---- global stats: all-reduce (mean, E[x^2]) sums ----
            # bn_aggr writes (mean, var); turn the var slot into E[x^2] in
            # place; the /N_CORES is folded into the post-AR chain
            pk = singles.tile([CO, 2], f32)
            sq = singles.tile([CO, 1], f32)
            nc.vector.bn_aggr(out=pk, in_=stats)
            nc.vector.tensor_mul(sq, pk[:, 0:1], pk[:, 0:1])
            nc.vector.tensor_add(pk[:, 1:2], pk[:, 1:2], sq)

            # AllGather ([128,2] per core -> [8*128,2]) has a lower floor
            # than AllReduce; the 8-way sum is done locally on DVE
            cc_in = dram.tile([CO, 2], f32)
            cc_out = dram.tile([N_CORES * CO, 2], f32)
            nc.sync.dma_start(out=cc_in, in_=pk)
            nc.gpsimd.collective_compute(
                "AllGather",
                mybir.AluOpType.bypass,
                replica_groups=[list(range(N_CORES))],
                ins=[cc_in[:].opt()],
                outs=[cc_out[:].opt()],
            )
            gsum = singles.tile([CO, N_CORES, 2], f32)
            for r in range(N_CORES):
                nc.sync.dma_start(
                    out=gsum[:, r, :], in_=cc_out[r * CO : (r + 1) * CO, :]
                )
            gst = singles.tile([CO, 2], f32)
            nc.vector.reduce_sum(
                out=gst,
                in_=gsum.rearrange("p r c -> p c r"),
                axis=mybir.AxisListType.X,
            )

            # gmean = sum/8 ; gvar = E2sum/8 - gmean^2
            # rstd = 1/sqrt(gvar + eps) ; shift = -gmean*rstd
            gmean = singles.tile([CO, 1], f32)
            gm2 = singles.tile([CO, 1], f32)
            gvar = singles.tile([CO, 1], f32)
            sd = singles.tile([CO, 1], f32)
            rstd = singles.tile([CO, 1], f32)
            shift = singles.tile([CO, 1], f32)
            eps_t = singles.tile([CO, 1], f32)
            nc.vector.memset(eps_t, EPS)
            nc.vector.tensor_scalar_mul(gmean, gst[:, 0:1], 1.0 / N_CORES)
            nc.vector.tensor_mul(gm2, gmean, gmean)
            nc.vector.tensor_scalar(
                out=gvar,
                in0=gst[:, 1:2],
                scalar1=1.0 / N_CORES,
                scalar2=gm2[:, 0:1],
                op0=mybir.AluOpType.mult,
                op1=mybir.AluOpType.subtract,
            )
            nc.scalar.activation(out=sd, in_=gvar, func=Sqrt, bias=eps_t[:, 0:1])
            nc.vector.reciprocal(rstd, sd)
            # shift = -gmean * rstd in one op
            nc.vector.tensor_scalar(
                out=shift,
                in0=gmean,
                scalar1=rstd[:, 0:1],
                scalar2=-1.0,
                op0=mybir.AluOpType.mult,
                op1=mybir.AluOpType.mult,
            )

            # ---- normalize (in place) + store ----
            # distribute the x*rstd+shift pass across DVE / ACT / GpSimd so
            # the store phase is DMA-bound instead of DVE-paced
            Ident = mybir.ActivationFunctionType.Identity
            # measured per-2048-chunk cost: DVE 1.34us, ACT 2.08us, GpSimd 4.2us
            ENG_SCHED = [0, 1, 0, 2, 0, 1, 0, 0, 1, 0, 2, 0, 1, 0, 0, 1]
            idx = 0
            for b in range(B_LOC):
                for c in range(L // XCH):
                    sl = conv_sb[:, b, c * XCH : (c + 1) * XCH]
                    eng = ENG_SCHED[idx % len(ENG_SCHED)]
                    if eng == 0:
                        nc.vector.tensor_scalar(
                            out=sl,
                            in0=sl,
                            scalar1=rstd[:, 0:1],
                            scalar2=shift[:, 0:1],
                            op0=mybir.AluOpType.mult,
                            op1=mybir.AluOpType.add,
                        )
                    elif eng == 1:
                        nc.scalar.activation(
                            out=sl,
                            in_=sl,
                            func=Ident,
                            bias=shift[:, 0:1],
                            scale=rstd[:, 0:1],
                        )
                    else:
                        nc.gpsimd.tensor_scalar(
                            out=sl,
                            in0=sl,
                            scalar1=rstd[:, 0:1],
                            scalar2=shift[:, 0:1],
                            op0=mybir.AluOpType.mult,
                            op1=mybir.AluOpType.add,
                        )
                    idx += 1
                    nc.sync.dma_start(
                        out=out[b, :, c * XCH : (c + 1) * XCH], in_=sl
                    )

    nc.compile()
    return nc


def _run(inputs, trace=False):
    from concourse import bass_utils

    x = np.ascontiguousarray(np.asarray(inputs["x"], dtype=np.float32))
    weight = np.ascontiguousarray(np.asarray(inputs["weight"], dtype=np.float32))

    if "nc" not in _CACHE:
        _CACHE["nc"] = _build_nc()
    nc = _CACHE["nc"]

    in_maps = [
        {"x": x[i * B_LOC : (i + 1) * B_LOC], "weight": weight}
        for i in range(N_CORES)
    ]
    res = bass_utils.run_bass_kernel_spmd(
        nc, in_maps, core_ids=list(range(N_CORES)), trace=trace
    )
    out = np.concatenate(
        [res.results[i]["out"] for i in range(N_CORES)], axis=0
    ).astype(np.float32)
    return out, res


def kernel(**inputs) -> np.ndarray:
    out, _ = _run(inputs, trace=False)
    return out

